# revision 40
# baseline (speedup 1.0000x reference)
import sys

if "/opt/trn_rl_repo" not in sys.path:
    sys.path.insert(0, "/opt/trn_rl_repo")

import numpy as np

from concourse import bacc, mybir, tile
from concourse import bass_utils as _bu
from concourse.bass_utils import run_bass_kernel_spmd



N_CORES = 8
B, C, H, W = 4096, 2, 64, 64
BPC = B // N_CORES          # 512 batches per core
NS = BPC // 16              # 32 supertiles of 16 maps each
NXCH = 4                    # x0 chunks of [128, 4096] per plane (8 supertiles)
NCHUNK = 8                  # data-loss chunks of [128, 4096] per tensor
CHUNK_F = 4096
GRID_D = 1.0 / (H - 1)
CLAMP_NEG_MIN = 27.6310211159  # -CLAMP_MIN
ESCALE = 512.0              # E is sent as fp8(E / ESCALE)
PSCALE = 8.0                # perm and a are sent scaled by 1/PSCALE so the
                            # fp8 u = (perm+a)(Eq p) stays within e4m3 range

# Engine for each data-loss chunk's subtract: "dve" | "gps" | "dr"
# ("dr" = DoubleRow matmul subtract on the tensor engine, squares from PSUM)
SUB_ENGINE = ["gps", "gps", "dve", "gps", "gps", "gps", "dve", "gps"]

F32 = mybir.dt.float32
BF16 = mybir.dt.bfloat16
FP8 = mybir.dt.float8e4


def _d1_matrix(n, d):
    m = np.zeros((n, n), dtype=np.float64)
    for i in range(1, n - 1):
        m[i, i - 1], m[i, i + 1] = -1.0, 1.0
    m[0, 0], m[0, 1], m[0, 2] = -3.0, 4.0, -1.0
    m[-1, -1], m[-1, -2], m[-1, -3] = 3.0, -4.0, 1.0
    return m / (2.0 * d)


def _d2_matrix(n, d):
    m = np.zeros((n, n), dtype=np.float64)
    for i in range(1, n - 1):
        m[i, i - 1], m[i, i], m[i, i + 1] = 1.0, -2.0, 1.0
    m[0, 0:4] = [2.0, -5.0, 4.0, -1.0]
    m[-1, -1], m[-1, -2], m[-1, -3], m[-1, -4] = 2.0, -5.0, 4.0, -1.0
    return m / (d * d)


def _build_consts():
    import ml_dtypes

    f8 = ml_dtypes.float8_e4m3
    bf = ml_dtypes.bfloat16

    d1 = _d1_matrix(H, GRID_D)
    d2 = _d2_matrix(H, GRID_D)
    e = -(d2 + d1.T @ d1)            # sum(perm*(E@p)) == -sum(perm*d2p) - sum(d1perm*d1p)
    g = d1[H - 1, :] - d1[0, :]      # Neumann-boundary row functional

    # E ships scaled into fp8; solve the boundary fold against the QUANTIZED
    # matrix so sum((perm + a 1^T) (.) Eq p) = sum(perm (.) Eq p)
    # + (g/ESCALE)^T rowsums(p) holds exactly for the matrix the PE uses.
    eq8 = (e / ESCALE).astype(f8)
    eq = eq8.astype(np.float64) * ESCALE
    a = np.linalg.lstsq(eq.T, g, rcond=None)[0]

    # lhsT for the E matmul: out = lhsT.T @ rhs must be blkdiag(Eq, Eq) @ rhs
    c_e = np.zeros((128, 128), dtype=f8)
    c_e[0:64, 0:64] = eq8.T
    c_e[64:128, 64:128] = eq8.T

    # Banded reduction weights for the DoubleRow pair-sum reduce. The banded
    # matmul runs in fp8 DoubleRow mode with W0 == W1, so each output column
    # sums a PAIR of u columns (same map, 32 apart): out free is 256 wide.
    # Parent layout [128, 2, 192] flattened: both 192-wide halves hold ones
    # at column 64 + p//64; slicing [64-2s : 192-2s] in the m dim puts the
    # ones at within-slice columns 2s, 2s+1 -> PSUM rows 2s, 2s+1.
    # (192-wide so the Ko dim stride is 192 bytes, a multiple of 16.)
    c_ones = np.zeros((128, 2, 192), dtype=f8)
    for p in range(128):
        c_ones[p, :, 64 + p // 64] = 1.0
    c_ones = c_ones.reshape(128, 384)

    # a-fold columns: +a for the h-direction plane, -a for the w-direction
    # plane (Neumann bc channel signs).
    c_a = np.zeros((128, 2), dtype=np.float32)
    c_a[:, 0] = np.tile(a, 2).astype(np.float32) / PSCALE
    c_a[:, 1] = -c_a[:, 0]

    # DoubleRow subtract weights: [128, 2, 128] flattened two-major so that
    # out = W0.T @ R0 + W1.T @ R1 = R0 - R1 with W0 = I, W1 = -I.
    c_w = np.zeros((128, 256), dtype=f8)
    c_w[:, 0:128] = np.eye(128, dtype=f8)
    c_w[:, 128:256] = -np.eye(128, dtype=f8)

    return {"cE": c_e, "cOnes": c_ones, "cA": c_a, "cW": c_w}


def _build_nc():
    nc = bacc.Bacc("TRN2", target_bir_lowering=False, debug=False)

    # x0 planes in supertile layout, fp8. xh = h-major (partition 64r+h,
    # free 512s+64j+w), xt = per-map transposed (partition 64r+w, free
    # 512s+64j+h). Chunk c holds supertiles 8c..8c+7; plane 0 = p, 1 = perm.
    xh = nc.dram_tensor("xh", [NXCH, 2, 128, CHUNK_F], FP8, kind="ExternalInput")
    xt = nc.dram_tensor("xt", [NXCH, 2, 128, CHUNK_F], FP8, kind="ExternalInput")
    # data-loss chunks: [mo | tg] concatenated along free
    md = nc.dram_tensor("md", [NCHUNK, 128, 2 * CHUNK_F], FP8, kind="ExternalInput")
    c_e = nc.dram_tensor("cE", [128, 128], FP8, kind="ExternalInput")
    c_ones = nc.dram_tensor("cOnes", [128, 384], FP8, kind="ExternalInput")
    c_a = nc.dram_tensor("cA", [128, 2], F32, kind="ExternalInput")
    c_w = nc.dram_tensor("cW", [128, 256], FP8, kind="ExternalInput")

    s_out = nc.dram_tensor("s", [64, 8], F32, kind="ExternalOutput")
    dstat_out = nc.dram_tensor("dstat", [128, 4 * NCHUNK], F32, kind="ExternalOutput")

    with tile.TileContext(nc) as tc:
        with (
            tc.tile_pool(name="consts", bufs=1) as cpool,
            tc.tile_pool(name="inp", bufs=12) as ipool,
            tc.tile_pool(name="dchunk", bufs=6) as dpool,
            tc.tile_pool(name="diff", bufs=3) as dfpool,
            tc.tile_pool(name="work", bufs=6) as wpool,
            tc.tile_pool(name="stats", bufs=1) as stpool,
            tc.tile_pool(name="pwork", bufs=3, space="PSUM") as pwpool,
            tc.tile_pool(name="paccum", bufs=1, space="PSUM") as papool,
        ):
            ce = cpool.tile([128, 128], FP8, tag="ce")
            cones = cpool.tile([128, 384], FP8, tag="cones")
            ca = cpool.tile([128, 2], F32, tag="ca")
            cw = cpool.tile([128, 256], FP8, tag="cw")

            sall = papool.tile([128, 256], F32, tag="sall")
            dstat = stpool.tile([128, 4 * NCHUNK], F32, tag="dstat")
            nc.gpsimd.memset(dstat[:], 0.0)

            # Deferred banded-reduce queue: (s, u_tile, half) entries are
            # emitted BANDED_LAG groups late so the tensor engine never
            # stalls waiting for the DVE's stt output (avoids tensor<->DVE
            # lockstep ping-pong).
            banded_q = []
            banded_emitted = 0
            BANDED_LAG = 4  # in queue entries (2 per group)

            cones3 = cones[:].rearrange("p (two m) -> p two m", two=2)

            def emit_banded(s, u_tile, h2, last=False):
                nonlocal banded_emitted
                lo, hi = 64 - 2 * s, 192 - 2 * s
                first = banded_emitted == 0
                banded_emitted += 1
                # DoubleRow with W0 == W1: out[m, n] accumulates the column
                # pair u[p, n] + u[p, n+256]. The host supertile layout puts
                # column (s, t, j, i) = w-split pairs 256 apart, so the pair
                # is same-map: free index 512s + 256t + 32j + i, w = 32t + i.
                rhs = (
                    u_tile[:, 512 * h2 : 512 * (h2 + 1)]
                    .rearrange("p (two n) -> p two n", two=2)
                )
                nc.tensor.matmul(
                    sall[:],
                    cones3[:, :, lo:hi],
                    rhs,
                    start=first,
                    stop=last,
                    skip_group_check=True,
                    perf_mode=mybir.MatmulPerfMode.DoubleRow,
                )

            # DMA launches are spread across sync/gpsimd/scalar (parallel
            # descriptor generation — a single engine serializes launches at
            # ~600ns each) and prefetched 2 chunks ahead.
            ptiles, mtiles = {}, {}

            def launch(k):
                plane, c = k % 2, k // 2
                src = xh if plane == 0 else xt
                p_t = ipool.tile([128, CHUNK_F], FP8, tag="p")
                perm_t = ipool.tile([128, CHUNK_F], FP8, tag="perm")
                nc.sync.dma_start(p_t[:], src[c, 0])
                nc.sync.dma_start(perm_t[:], src[c, 1])
                mt = dpool.tile([128, 2 * CHUNK_F], FP8, tag="mt")
                nc.sync.dma_start(mt[:], md[k])
                ptiles[k] = (p_t, perm_t)
                mtiles[k] = mt

            # consts first — they gate the tensor stream and must not queue
            # behind megabyte bulk transfers
            nc.sync.dma_start(ce[:], c_e[:])
            nc.sync.dma_start(cw[:], c_w[:])
            nc.sync.dma_start(cones[:], c_ones[:])
            nc.sync.dma_start(ca[:], c_a[:])
            launch(0)
            launch(1)

            for k in range(NCHUNK):
                plane = k % 2          # 0: h-major (xh), 1: w-major (xt)
                c = k // 2             # x0 chunk index
                if k + 2 < NCHUNK:
                    launch(k + 2)
                p_t, perm_t = ptiles.pop(k)
                mt = mtiles.pop(k)

                # data loss for this chunk
                sub = SUB_ENGINE[k]
                if sub == "dr":
                    # DoubleRow matmul subtract into PSUM; scalar squares from
                    # PSUM per 1024 columns.
                    for g in range(4):
                        dp = pwpool.tile([128, 1024], F32, tag="ep")
                        mt3 = mt[:].rearrange("p (two n) -> p two n", two=2)
                        for h2 in range(2):
                            t = 2 * g + h2
                            rhs3 = mt3[:, :, 512 * t : 512 * (t + 1)]
                            nc.tensor.matmul(
                                dp[:, 512 * h2 : 512 * (h2 + 1)],
                                cw[:].rearrange("p (two m) -> p two m", two=2),
                                rhs3,
                                start=True,
                                stop=True,
                                perf_mode=mybir.MatmulPerfMode.DoubleRow,
                            )
                        junk = dfpool.tile([128, 1024], BF16, tag="df")
                        nc.scalar.activation(
                            junk[:],
                            dp[:],
                            mybir.ActivationFunctionType.Square,
                            accum_out=dstat[:, 4 * k + g : 4 * k + g + 1],
                        )
                else:
                    df = dfpool.tile([128, CHUNK_F], BF16, tag="df")
                    eng = nc.vector if sub == "dve" else nc.gpsimd
                    eng.tensor_sub(
                        df[:], mt[:, 0:CHUNK_F], mt[:, CHUNK_F : 2 * CHUNK_F]
                    )
                    # split into halves so the scalar queue never blocks the
                    # shared PSUM pool for a full 4µs activation
                    for hh in range(2):
                        nc.scalar.activation(
                            df[:, 2048 * hh : 2048 * (hh + 1)],
                            df[:, 2048 * hh : 2048 * (hh + 1)],
                            mybir.ActivationFunctionType.Square,
                            accum_out=dstat[:, 4 * k + hh : 4 * k + hh + 1],
                        )

                # residual: 8 supertiles per chunk in pairs; both planes
                # accumulate into the same PSUM rows (identical batch mapping).
                for g in range(4):
                    ep = pwpool.tile([128, 1024], F32, tag="ep")
                    for h2 in range(2):
                        t = 2 * g + h2
                        nc.tensor.matmul(
                            ep[:, 512 * h2 : 512 * (h2 + 1)],
                            ce[:],
                            p_t[:, 512 * t : 512 * (t + 1)],
                            start=True,
                            stop=True,
                        )
                    u = wpool.tile([128, 1024], FP8, tag="u")
                    nc.vector.scalar_tensor_tensor(
                        u[:],
                        perm_t[:, 1024 * g : 1024 * (g + 1)],
                        ca[:, plane : plane + 1],
                        ep[:],
                        op0=mybir.AluOpType.add,
                        op1=mybir.AluOpType.mult,
                    )
                    for h2 in range(2):
                        banded_q.append((8 * c + 2 * g + h2, u, h2))
                    while len(banded_q) > BANDED_LAG:
                        emit_banded(*banded_q.pop(0))

            while banded_q:
                ent = banded_q.pop(0)
                emit_banded(*ent, last=not banded_q)

            s_t = stpool.tile([64, 8], F32, tag="st")
            nc.vector.reduce_sum(
                s_t[:],
                sall[0:64, :].rearrange("p (j w) -> p j w", j=8),
                axis=mybir.AxisListType.X,
            )  # 32 pair-columns per map
            nc.sync.dma_start(s_out[:], s_t[:])
            nc.sync.dma_start(dstat_out[:], dstat[:])

    nc.compile()
    return nc


_NC = None
_CONSTS = None
LAST_RESULTS = None


def kernel(model_out, target, x0_hat, var, _trace=False, _trace_kwargs=None):
    global _NC, _CONSTS, LAST_RESULTS
    if _NC is None:
        _CONSTS = _build_consts()
        _NC = _build_nc()

    import ml_dtypes

    f8 = ml_dtypes.float8_e4m3
    model_out = np.asarray(model_out).astype(f8)
    target = np.asarray(target).astype(f8)
    x0f = np.asarray(x0_hat, dtype=np.float32).copy()
    x0f[:, 1] *= 1.0 / PSCALE  # keep fp8 u in range
    x0_hat = x0f.astype(f8)
    var = np.asarray(var, dtype=np.float32)

    in_maps = []
    for cid in range(N_CORES):
        lo, hi = cid * BPC, (cid + 1) * BPC
        # (s, r, j, ch, h, t, i) with batch = 16s + 8r + j and w = 32t + i;
        # free index is 512s + 256t + 32j + i so same-map column pairs sit
        # 256 apart (DoubleRow banded pair-sum).
        x0c = x0_hat[lo:hi].reshape(NS, 2, 8, 2, H, 2, 32)
        # h-major: [ch, 64r+h, 512s+256t+32j+i]
        xh_arr = (
            x0c.transpose(3, 1, 4, 0, 5, 2, 6)
            .reshape(2, 128, NXCH, CHUNK_F)
            .transpose(2, 0, 1, 3)
            .copy()
        )
        # w-major (per-map transpose): [ch, 64r+w, 512s+256th+32j+ih],
        # h = 32*th + ih
        x0t = x0_hat[lo:hi].reshape(NS, 2, 8, 2, 2, 32, W)
        xt_arr = (
            x0t.transpose(3, 1, 6, 0, 4, 2, 5)
            .reshape(2, 128, NXCH, CHUNK_F)
            .transpose(2, 0, 1, 3)
            .copy()
        )
        md_arr = np.concatenate(
            [
                model_out[lo:hi].reshape(NCHUNK, 128, CHUNK_F),
                target[lo:hi].reshape(NCHUNK, 128, CHUNK_F),
            ],
            axis=-1,
        )
        in_maps.append({"xh": xh_arr, "xt": xt_arr, "md": md_arr, **_CONSTS})

    kwargs = {}
    if _trace:
        kwargs["trace"] = True
        if _trace_kwargs:
            kwargs.update(_trace_kwargs)
    res = run_bass_kernel_spmd(_NC, in_maps, list(range(N_CORES)), **kwargs)
    LAST_RESULTS = res

    data_sum = 0.0
    nll_sum = 0.0
    for cid in range(N_CORES):
        out = res.results[cid]
        s1 = out["s"].astype(np.float64)         # [64, 8]
        dstat = out["dstat"].astype(np.float64)  # [128, 32]

        # s1[2s+r, j] -> batch 16s + 8r + j
        r = s1.reshape(NS, 2, 8).reshape(BPC) * (ESCALE * PSCALE / (H * W * 3.0))

        v = var[cid * BPC : (cid + 1) * BPC].astype(np.float64)
        nll = np.minimum(0.5 * r * r / v, CLAMP_NEG_MIN)
        nll_sum += nll.sum()
        data_sum += dstat.sum()

    loss = data_sum / (B * C * H * W) + nll_sum / B
    return np.float32(loss)


# revision 53
# speedup vs baseline: 1.0572x; 1.0572x over previous
import sys

if "/opt/trn_rl_repo" not in sys.path:
    sys.path.insert(0, "/opt/trn_rl_repo")

import numpy as np

from concourse import bacc, mybir, tile
from concourse import bass_utils as _bu
from concourse.bass_utils import run_bass_kernel_spmd



N_CORES = 8
B, C, H, W = 4096, 2, 64, 64
BPC = B // N_CORES          # 512 batches per core
NS = BPC // 16              # 32 supertiles of 16 maps each
NXCH = 4                    # x0 chunks of [128, 4096] per plane (8 supertiles)
NCHUNK = 8                  # data-loss chunks of [128, 4096] per tensor
CHUNK_F = 4096
GRID_D = 1.0 / (H - 1)
CLAMP_NEG_MIN = 27.6310211159  # -CLAMP_MIN
ESCALE = 512.0              # E is sent as fp8(E / ESCALE)
PSCALE = 8.0                # perm and a are sent scaled by 1/PSCALE so the
                            # fp8 u = (perm+a)(Eq p) stays within e4m3 range

# Engine for each data-loss chunk's subtract: "dve" | "gps" | "dr"
# ("dr" = DoubleRow matmul subtract on the tensor engine, squares from PSUM)
SUB_ENGINE = ["gps", "dve", "gps", "gps", "gps", "dve", "gps", "gps"]

F32 = mybir.dt.float32
BF16 = mybir.dt.bfloat16
FP8 = mybir.dt.float8e4


def _d1_matrix(n, d):
    m = np.zeros((n, n), dtype=np.float64)
    for i in range(1, n - 1):
        m[i, i - 1], m[i, i + 1] = -1.0, 1.0
    m[0, 0], m[0, 1], m[0, 2] = -3.0, 4.0, -1.0
    m[-1, -1], m[-1, -2], m[-1, -3] = 3.0, -4.0, 1.0
    return m / (2.0 * d)


def _d2_matrix(n, d):
    m = np.zeros((n, n), dtype=np.float64)
    for i in range(1, n - 1):
        m[i, i - 1], m[i, i], m[i, i + 1] = 1.0, -2.0, 1.0
    m[0, 0:4] = [2.0, -5.0, 4.0, -1.0]
    m[-1, -1], m[-1, -2], m[-1, -3], m[-1, -4] = 2.0, -5.0, 4.0, -1.0
    return m / (d * d)


def _build_consts():
    import ml_dtypes

    f8 = ml_dtypes.float8_e4m3
    bf = ml_dtypes.bfloat16

    d1 = _d1_matrix(H, GRID_D)
    d2 = _d2_matrix(H, GRID_D)
    e = -(d2 + d1.T @ d1)            # sum(perm*(E@p)) == -sum(perm*d2p) - sum(d1perm*d1p)
    g = d1[H - 1, :] - d1[0, :]      # Neumann-boundary row functional

    # E ships scaled into fp8; solve the boundary fold against the QUANTIZED
    # matrix so sum((perm + a 1^T) (.) Eq p) = sum(perm (.) Eq p)
    # + (g/ESCALE)^T rowsums(p) holds exactly for the matrix the PE uses.
    eq8 = (e / ESCALE).astype(f8)
    eq = eq8.astype(np.float64) * ESCALE
    a = np.linalg.lstsq(eq.T, g, rcond=None)[0]

    # lhsT for the E matmul: out = lhsT.T @ rhs must be blkdiag(Eq, Eq) @ rhs
    c_e = np.zeros((128, 128), dtype=f8)
    c_e[0:64, 0:64] = eq8.T
    c_e[64:128, 64:128] = eq8.T

    # Banded reduction weights for the DoubleRow pair-sum reduce. The banded
    # matmul runs in fp8 DoubleRow mode with W0 == W1, so each output column
    # sums a PAIR of u columns (same map, 32 apart): out free is 256 wide.
    # Parent layout [128, 2, 192] flattened: both 192-wide halves hold ones
    # at column 64 + p//64; slicing [64-2s : 192-2s] in the m dim puts the
    # ones at within-slice columns 2s, 2s+1 -> PSUM rows 2s, 2s+1.
    # (192-wide so the Ko dim stride is 192 bytes, a multiple of 16.)
    c_ones = np.zeros((128, 2, 192), dtype=f8)
    for p in range(128):
        c_ones[p, :, 64 + p // 64] = 1.0
    c_ones = c_ones.reshape(128, 384)

    # a-fold columns: +a for the h-direction plane, -a for the w-direction
    # plane (Neumann bc channel signs).
    c_a = np.zeros((128, 3), dtype=np.float32)
    c_a[:, 0] = np.tile(a, 2).astype(np.float32) / PSCALE
    c_a[:, 1] = -c_a[:, 0]
    c_a[:, 2] = 1.0

    # DoubleRow subtract weights: [128, 2, 128] flattened two-major so that
    # out = W0.T @ R0 + W1.T @ R1 = R0 - R1 with W0 = I, W1 = -I.
    c_w = np.zeros((128, 256), dtype=f8)
    c_w[:, 0:128] = np.eye(128, dtype=f8)
    c_w[:, 128:256] = -np.eye(128, dtype=f8)

    return {"cE": c_e, "cOnes": c_ones, "cA": c_a, "cW": c_w}


def _build_nc():
    nc = bacc.Bacc("TRN2", target_bir_lowering=False, debug=False)

    # x0 planes in supertile layout, fp8. xh = h-major (partition 64r+h,
    # free 512s+64j+w), xt = per-map transposed (partition 64r+w, free
    # 512s+64j+h). Chunk c holds supertiles 8c..8c+7; plane 0 = p, 1 = perm.
    xh = nc.dram_tensor("xh", [NXCH, 2, 128, CHUNK_F], FP8, kind="ExternalInput")
    xt = nc.dram_tensor("xt", [NXCH, 2, 128, CHUNK_F], FP8, kind="ExternalInput")
    # data-loss chunks: [mo | tg] concatenated along free
    md = nc.dram_tensor("md", [NCHUNK, 128, 2 * CHUNK_F], FP8, kind="ExternalInput")
    c_e = nc.dram_tensor("cE", [128, 128], FP8, kind="ExternalInput")
    c_ones = nc.dram_tensor("cOnes", [128, 384], FP8, kind="ExternalInput")
    c_a = nc.dram_tensor("cA", [128, 3], F32, kind="ExternalInput")
    use_dr = "dr" in SUB_ENGINE
    c_w = (
        nc.dram_tensor("cW", [128, 256], FP8, kind="ExternalInput")
        if use_dr
        else None
    )

    s_out = nc.dram_tensor("s", [64, 8], F32, kind="ExternalOutput")
    dstat_out = nc.dram_tensor(
        "dstat", [128, 4 * NCHUNK + 1], F32, kind="ExternalOutput"
    )

    with tile.TileContext(nc) as tc:
        with (
            tc.tile_pool(name="consts", bufs=1) as cpool,
            tc.tile_pool(name="inp", bufs=12) as ipool,
            tc.tile_pool(name="dchunk", bufs=6) as dpool,
            tc.tile_pool(name="diff", bufs=3) as dfpool,
            tc.tile_pool(name="work", bufs=6) as wpool,
            tc.tile_pool(name="stats", bufs=1) as stpool,
            tc.tile_pool(name="pwork", bufs=3, space="PSUM") as pwpool,
            tc.tile_pool(name="paccum", bufs=1, space="PSUM") as papool,
        ):
            ce = cpool.tile([128, 128], FP8, tag="ce")
            cones = cpool.tile([128, 384], FP8, tag="cones")
            ca = cpool.tile([128, 3], F32, tag="ca")
            cw = cpool.tile([128, 256], FP8, tag="cw") if use_dr else None

            sall = papool.tile([128, 256], F32, tag="sall")
            dstat = stpool.tile([128, 4 * NCHUNK + 1], F32, tag="dstat")
            nc.gpsimd.memset(dstat[:], 0.0)

            # the BIR verifier requires every pre-registered const AP to have
            # a reader; touch the ones nothing else consumes
            for cdt, cval in [
                (mybir.dt.float32, 1.0),
                (mybir.dt.bfloat16, 1.0),
                (mybir.dt.uint8, 127),
            ]:
                nc.gpsimd.tensor_copy(
                    dstat[:, 4 * NCHUNK : 4 * NCHUNK + 1],
                    nc.const_aps.aps[(cdt, cval)],
                )

            # Deferred banded-reduce queue: (s, u_tile, half) entries are
            # emitted BANDED_LAG groups late so the tensor engine never
            # stalls waiting for the DVE's stt output (avoids tensor<->DVE
            # lockstep ping-pong).
            banded_q = []
            banded_emitted = 0
            BANDED_LAG = 4  # in queue entries (2 per group)

            cones3 = cones[:].rearrange("p (two m) -> p two m", two=2)

            def emit_banded(s, u_tile, h2, last=False):
                nonlocal banded_emitted
                lo, hi = 64 - 2 * s, 192 - 2 * s
                first = banded_emitted == 0
                banded_emitted += 1
                # DoubleRow with W0 == W1: out[m, n] accumulates the column
                # pair u[p, n] + u[p, n+256]. The host supertile layout puts
                # column (s, t, j, i) = w-split pairs 256 apart, so the pair
                # is same-map: free index 512s + 256t + 32j + i, w = 32t + i.
                rhs = (
                    u_tile[:, 512 * h2 : 512 * (h2 + 1)]
                    .rearrange("p (two n) -> p two n", two=2)
                )
                nc.tensor.matmul(
                    sall[:],
                    cones3[:, :, lo:hi],
                    rhs,
                    start=first,
                    stop=last,
                    skip_group_check=True,
                    perf_mode=mybir.MatmulPerfMode.DoubleRow,
                )

            # DMA launches are spread across sync/gpsimd/scalar (parallel
            # descriptor generation — a single engine serializes launches at
            # ~600ns each) and prefetched 2 chunks ahead.
            ptiles, mtiles = {}, {}

            def launch(k):
                plane, c = k % 2, k // 2
                src = xh if plane == 0 else xt
                p_t = ipool.tile([128, CHUNK_F], FP8, tag="p")
                perm_t = ipool.tile([128, CHUNK_F], FP8, tag="perm")
                nc.sync.dma_start(p_t[:], src[c, 0])
                nc.sync.dma_start(perm_t[:], src[c, 1])
                mt = dpool.tile([128, 2 * CHUNK_F], FP8, tag="mt")
                nc.sync.dma_start(mt[:], md[k])
                ptiles[k] = (p_t, perm_t)
                mtiles[k] = mt

            # chunk 0's p/perm lead (they gate the first E-mm + stt), then the
            # tiny consts, then the rest — order matters: DMA queues are FIFO
            p0 = ipool.tile([128, CHUNK_F], FP8, tag="p")
            perm0 = ipool.tile([128, CHUNK_F], FP8, tag="perm")
            nc.sync.dma_start(p0[:], xh[0, 0])
            nc.sync.dma_start(perm0[:], xh[0, 1])
            nc.sync.dma_start(ce[:], c_e[:])
            if use_dr:
                nc.sync.dma_start(cw[:], c_w[:])
            nc.sync.dma_start(cones[:], c_ones[:])
            nc.sync.dma_start(ca[:], c_a[:])
            mt0 = dpool.tile([128, 2 * CHUNK_F], FP8, tag="mt")
            nc.sync.dma_start(mt0[:], md[0])
            ptiles[0] = (p0, perm0)
            mtiles[0] = mt0
            launch(1)

            for k in range(NCHUNK):
                plane = k % 2          # 0: h-major (xh), 1: w-major (xt)
                c = k // 2             # x0 chunk index
                if k + 2 < NCHUNK:
                    launch(k + 2)
                p_t, perm_t = ptiles.pop(k)
                mt = mtiles.pop(k)

                # data loss for this chunk
                sub = SUB_ENGINE[k]
                if sub == "dr":
                    # DoubleRow matmul subtract into PSUM; scalar squares from
                    # PSUM per 1024 columns.
                    for g in range(4):
                        dp = pwpool.tile([128, 1024], F32, tag="ep")
                        mt3 = mt[:].rearrange("p (two n) -> p two n", two=2)
                        for h2 in range(2):
                            t = 2 * g + h2
                            rhs3 = mt3[:, :, 512 * t : 512 * (t + 1)]
                            nc.tensor.matmul(
                                dp[:, 512 * h2 : 512 * (h2 + 1)],
                                cw[:].rearrange("p (two m) -> p two m", two=2),
                                rhs3,
                                start=True,
                                stop=True,
                                perf_mode=mybir.MatmulPerfMode.DoubleRow,
                            )
                        junk = dfpool.tile([128, 1024], BF16, tag="df")
                        nc.scalar.activation(
                            junk[:],
                            dp[:],
                            mybir.ActivationFunctionType.Square,
                            accum_out=dstat[:, 4 * k + g : 4 * k + g + 1],
                        )
                else:
                    # DVE runs the subtract as scalar_tensor_tensor (faster
                    # than tensor_tensor there); Pool can't run stt.
                    df = dfpool.tile([128, CHUNK_F], FP8, tag="df")
                    if sub == "dve":
                        nc.vector.scalar_tensor_tensor(
                            df[:],
                            mt[:, 0:CHUNK_F],
                            ca[:, 2:3],
                            mt[:, CHUNK_F : 2 * CHUNK_F],
                            op0=mybir.AluOpType.mult,
                            op1=mybir.AluOpType.subtract,
                        )
                    else:
                        nc.gpsimd.tensor_sub(
                            df[:], mt[:, 0:CHUNK_F], mt[:, CHUNK_F : 2 * CHUNK_F]
                        )
                    # split into halves so the scalar queue never blocks the
                    # shared PSUM pool for a full 4µs activation
                    for hh in range(2):
                        nc.scalar.activation(
                            df[:, 2048 * hh : 2048 * (hh + 1)],
                            df[:, 2048 * hh : 2048 * (hh + 1)],
                            mybir.ActivationFunctionType.Square,
                            accum_out=dstat[:, 4 * k + hh : 4 * k + hh + 1],
                        )

                # residual: 8 supertiles per chunk in pairs; both planes
                # accumulate into the same PSUM rows (identical batch mapping).
                for g in range(4):
                    ep = pwpool.tile([128, 1024], F32, tag="ep")
                    for h2 in range(2):
                        t = 2 * g + h2
                        nc.tensor.matmul(
                            ep[:, 512 * h2 : 512 * (h2 + 1)],
                            ce[:],
                            p_t[:, 512 * t : 512 * (t + 1)],
                            start=True,
                            stop=True,
                        )
                    u = wpool.tile([128, 1024], FP8, tag="u")
                    nc.vector.scalar_tensor_tensor(
                        u[:],
                        perm_t[:, 1024 * g : 1024 * (g + 1)],
                        ca[:, plane : plane + 1],
                        ep[:],
                        op0=mybir.AluOpType.add,
                        op1=mybir.AluOpType.mult,
                    )
                    for h2 in range(2):
                        banded_q.append((8 * c + 2 * g + h2, u, h2))
                    while len(banded_q) > BANDED_LAG:
                        emit_banded(*banded_q.pop(0))

            while banded_q:
                ent = banded_q.pop(0)
                emit_banded(*ent, last=not banded_q)

            s_t = stpool.tile([64, 8], F32, tag="st")
            nc.vector.reduce_sum(
                s_t[:],
                sall[0:64, :].rearrange("p (j w) -> p j w", j=8),
                axis=mybir.AxisListType.X,
            )  # 32 pair-columns per map
            nc.sync.dma_start(s_out[:], s_t[:])
            nc.sync.dma_start(dstat_out[:], dstat[:])

    nc.compile()
    return nc


_NC = None
_CONSTS = None
LAST_RESULTS = None


def kernel(model_out, target, x0_hat, var, _trace=False, _trace_kwargs=None):
    global _NC, _CONSTS, LAST_RESULTS
    if _NC is None:
        _CONSTS = _build_consts()
        _NC = _build_nc()

    import ml_dtypes

    f8 = ml_dtypes.float8_e4m3
    model_out = np.asarray(model_out).astype(f8)
    target = np.asarray(target).astype(f8)
    x0f = np.asarray(x0_hat, dtype=np.float32).copy()
    x0f[:, 1] *= 1.0 / PSCALE  # keep fp8 u in range
    x0_hat = x0f.astype(f8)
    var = np.asarray(var, dtype=np.float32)

    in_maps = []
    for cid in range(N_CORES):
        lo, hi = cid * BPC, (cid + 1) * BPC
        # (s, r, j, ch, h, t, i) with batch = 16s + 8r + j and w = 32t + i;
        # free index is 512s + 256t + 32j + i so same-map column pairs sit
        # 256 apart (DoubleRow banded pair-sum).
        x0c = x0_hat[lo:hi].reshape(NS, 2, 8, 2, H, 2, 32)
        # h-major: [ch, 64r+h, 512s+256t+32j+i]
        xh_arr = (
            x0c.transpose(3, 1, 4, 0, 5, 2, 6)
            .reshape(2, 128, NXCH, CHUNK_F)
            .transpose(2, 0, 1, 3)
            .copy()
        )
        # w-major (per-map transpose): [ch, 64r+w, 512s+256th+32j+ih],
        # h = 32*th + ih
        x0t = x0_hat[lo:hi].reshape(NS, 2, 8, 2, 2, 32, W)
        xt_arr = (
            x0t.transpose(3, 1, 6, 0, 4, 2, 5)
            .reshape(2, 128, NXCH, CHUNK_F)
            .transpose(2, 0, 1, 3)
            .copy()
        )
        md_arr = np.concatenate(
            [
                model_out[lo:hi].reshape(NCHUNK, 128, CHUNK_F),
                target[lo:hi].reshape(NCHUNK, 128, CHUNK_F),
            ],
            axis=-1,
        )
        consts = dict(_CONSTS)
        if "dr" not in SUB_ENGINE:
            consts.pop("cW", None)
        in_maps.append({"xh": xh_arr, "xt": xt_arr, "md": md_arr, **consts})

    kwargs = {}
    if _trace:
        kwargs["trace"] = True
        if _trace_kwargs:
            kwargs.update(_trace_kwargs)
    res = run_bass_kernel_spmd(_NC, in_maps, list(range(N_CORES)), **kwargs)
    LAST_RESULTS = res

    data_sum = 0.0
    nll_sum = 0.0
    for cid in range(N_CORES):
        out = res.results[cid]
        s1 = out["s"].astype(np.float64)         # [64, 8]
        dstat = out["dstat"][:, : 4 * NCHUNK].astype(np.float64)

        # s1[2s+r, j] -> batch 16s + 8r + j
        r = s1.reshape(NS, 2, 8).reshape(BPC) * (ESCALE * PSCALE / (H * W * 3.0))

        v = var[cid * BPC : (cid + 1) * BPC].astype(np.float64)
        nll = np.minimum(0.5 * r * r / v, CLAMP_NEG_MIN)
        nll_sum += nll.sum()
        data_sum += dstat.sum()

    loss = data_sum / (B * C * H * W) + nll_sum / B
    return np.float32(loss)


# revision 58
# speedup vs baseline: 1.0689x; 1.0111x over previous
import sys

if "/opt/trn_rl_repo" not in sys.path:
    sys.path.insert(0, "/opt/trn_rl_repo")

import numpy as np

from concourse import bacc, mybir, tile
from concourse import bass_utils as _bu
from concourse.bass_utils import run_bass_kernel_spmd



N_CORES = 8
B, C, H, W = 4096, 2, 64, 64
BPC = B // N_CORES          # 512 batches per core
NS = BPC // 16              # 32 supertiles of 16 maps each
NXCH = 4                    # x0 chunks of [128, 4096] per plane (8 supertiles)
NCHUNK = 8                  # data-loss chunks of [128, 4096] per tensor
CHUNK_F = 4096
GRID_D = 1.0 / (H - 1)
CLAMP_NEG_MIN = 27.6310211159  # -CLAMP_MIN
ESCALE = 512.0              # E is sent as fp8(E / ESCALE)
PSCALE = 8.0                # perm and a are sent scaled by 1/PSCALE so the
                            # fp8 u = (perm+a)(Eq p) stays within e4m3 range

# Engine for each data-loss chunk's subtract: "dve" | "gps" | "dr"
# ("dr" = DoubleRow matmul subtract on the tensor engine, squares from PSUM)
SUB_ENGINE = ["dr", "gps", "dr", "gps", "dr", "gps", "dr", "gps"]

F32 = mybir.dt.float32
BF16 = mybir.dt.bfloat16
FP8 = mybir.dt.float8e4


def _d1_matrix(n, d):
    m = np.zeros((n, n), dtype=np.float64)
    for i in range(1, n - 1):
        m[i, i - 1], m[i, i + 1] = -1.0, 1.0
    m[0, 0], m[0, 1], m[0, 2] = -3.0, 4.0, -1.0
    m[-1, -1], m[-1, -2], m[-1, -3] = 3.0, -4.0, 1.0
    return m / (2.0 * d)


def _d2_matrix(n, d):
    m = np.zeros((n, n), dtype=np.float64)
    for i in range(1, n - 1):
        m[i, i - 1], m[i, i], m[i, i + 1] = 1.0, -2.0, 1.0
    m[0, 0:4] = [2.0, -5.0, 4.0, -1.0]
    m[-1, -1], m[-1, -2], m[-1, -3], m[-1, -4] = 2.0, -5.0, 4.0, -1.0
    return m / (d * d)


def _build_consts():
    import ml_dtypes

    f8 = ml_dtypes.float8_e4m3
    bf = ml_dtypes.bfloat16

    d1 = _d1_matrix(H, GRID_D)
    d2 = _d2_matrix(H, GRID_D)
    e = -(d2 + d1.T @ d1)            # sum(perm*(E@p)) == -sum(perm*d2p) - sum(d1perm*d1p)
    g = d1[H - 1, :] - d1[0, :]      # Neumann-boundary row functional

    # E ships scaled into fp8; solve the boundary fold against the QUANTIZED
    # matrix so sum((perm + a 1^T) (.) Eq p) = sum(perm (.) Eq p)
    # + (g/ESCALE)^T rowsums(p) holds exactly for the matrix the PE uses.
    eq8 = (e / ESCALE).astype(f8)
    eq = eq8.astype(np.float64) * ESCALE
    a = np.linalg.lstsq(eq.T, g, rcond=None)[0]

    # lhsT for the E matmul: out = lhsT.T @ rhs must be blkdiag(Eq, Eq) @ rhs
    c_e = np.zeros((128, 128), dtype=f8)
    c_e[0:64, 0:64] = eq8.T
    c_e[64:128, 64:128] = eq8.T

    # Banded reduction weights for the DoubleRow pair-sum reduce. The banded
    # matmul runs in fp8 DoubleRow mode with W0 == W1, so each output column
    # sums a PAIR of u columns (same map, 32 apart): out free is 256 wide.
    # Parent layout [128, 2, 192] flattened: both 192-wide halves hold ones
    # at column 64 + p//64; slicing [64-2s : 192-2s] in the m dim puts the
    # ones at within-slice columns 2s, 2s+1 -> PSUM rows 2s, 2s+1.
    # (192-wide so the Ko dim stride is 192 bytes, a multiple of 16.)
    c_ones = np.zeros((128, 2, 192), dtype=f8)
    for p in range(128):
        c_ones[p, :, 64 + p // 64] = 1.0
    c_ones = c_ones.reshape(128, 384)

    # a-fold columns: +a for the h-direction plane, -a for the w-direction
    # plane (Neumann bc channel signs).
    c_a = np.zeros((128, 3), dtype=np.float32)
    c_a[:, 0] = np.tile(a, 2).astype(np.float32) / PSCALE
    c_a[:, 1] = -c_a[:, 0]
    c_a[:, 2] = 1.0

    # DoubleRow subtract weights: [128, 2, 128] flattened two-major so that
    # out = W0.T @ R0 + W1.T @ R1 = R0 - R1 with W0 = I, W1 = -I.
    c_w = np.zeros((128, 256), dtype=f8)
    c_w[:, 0:128] = np.eye(128, dtype=f8)
    c_w[:, 128:256] = -np.eye(128, dtype=f8)

    return {"cE": c_e, "cOnes": c_ones, "cA": c_a, "cW": c_w}


def _build_nc():
    nc = bacc.Bacc("TRN2", target_bir_lowering=False, debug=False)

    # x0 planes in supertile layout, fp8. xh = h-major (partition 64r+h,
    # free 512s+64j+w), xt = per-map transposed (partition 64r+w, free
    # 512s+64j+h). Chunk c holds supertiles 8c..8c+7; plane 0 = p, 1 = perm.
    xh = nc.dram_tensor("xh", [NXCH, 2, 128, CHUNK_F], FP8, kind="ExternalInput")
    xt = nc.dram_tensor("xt", [NXCH, 2, 128, CHUNK_F], FP8, kind="ExternalInput")
    # data-loss chunks: [mo | tg] concatenated along free
    md = nc.dram_tensor("md", [NCHUNK, 128, 2 * CHUNK_F], FP8, kind="ExternalInput")
    c_e = nc.dram_tensor("cE", [128, 128], FP8, kind="ExternalInput")
    c_ones = nc.dram_tensor("cOnes", [128, 384], FP8, kind="ExternalInput")
    c_a = nc.dram_tensor("cA", [128, 3], F32, kind="ExternalInput")
    use_dr = "dr" in SUB_ENGINE
    c_w = (
        nc.dram_tensor("cW", [128, 256], FP8, kind="ExternalInput")
        if use_dr
        else None
    )

    s_out = nc.dram_tensor("s", [64, 8], F32, kind="ExternalOutput")
    dstat_out = nc.dram_tensor(
        "dstat", [128, 4 * NCHUNK + 1], F32, kind="ExternalOutput"
    )

    with tile.TileContext(nc) as tc:
        with (
            tc.tile_pool(name="consts", bufs=1) as cpool,
            tc.tile_pool(name="inp", bufs=12) as ipool,
            tc.tile_pool(name="dchunk", bufs=6) as dpool,
            tc.tile_pool(name="diff", bufs=3) as dfpool,
            tc.tile_pool(name="work", bufs=6) as wpool,
            tc.tile_pool(name="stats", bufs=1) as stpool,
            tc.tile_pool(name="pwork", bufs=3, space="PSUM") as pwpool,
            tc.tile_pool(name="paccum", bufs=1, space="PSUM") as papool,
        ):
            ce = cpool.tile([128, 128], FP8, tag="ce")
            cones = cpool.tile([128, 384], FP8, tag="cones")
            ca = cpool.tile([128, 3], F32, tag="ca")
            if use_dr:
                cw = cpool.tile([128, 256], FP8, tag="cw")
            else:
                cw = None

            sall = papool.tile([128, 256], F32, tag="sall")
            dstat = stpool.tile([128, 4 * NCHUNK + 1], F32, tag="dstat")
            nc.gpsimd.memset(dstat[:], 0.0)

            # the BIR verifier requires every pre-registered const AP to have
            # a reader; touch the ones nothing else consumes
            for cdt, cval in [
                (mybir.dt.float32, 1.0),
                (mybir.dt.bfloat16, 1.0),
                (mybir.dt.uint8, 127),
            ]:
                nc.gpsimd.tensor_copy(
                    dstat[:, 4 * NCHUNK : 4 * NCHUNK + 1],
                    nc.const_aps.aps[(cdt, cval)],
                )

            # Deferred banded-reduce queue: (s, u_tile, half) entries are
            # emitted BANDED_LAG groups late so the tensor engine never
            # stalls waiting for the DVE's stt output (avoids tensor<->DVE
            # lockstep ping-pong).
            banded_q = []
            banded_emitted = 0
            BANDED_LAG = 4  # in queue entries (2 per group)

            # Deferred DoubleRow-subtract queue: (mt_tile, k, q) quarters.
            dr_q = []

            def emit_dr(mt_tile, kk, q):
                dp = pwpool.tile([128, 1024], F32, tag="ep")
                mt3 = mt_tile[:].rearrange("p (two n) -> p two n", two=2)
                for h2 in range(2):
                    t = 2 * q + h2
                    nc.tensor.matmul(
                        dp[:, 512 * h2 : 512 * (h2 + 1)],
                        cw[:].rearrange("p (two m) -> p two m", two=2),
                        mt3[:, :, 512 * t : 512 * (t + 1)],
                        start=True,
                        stop=True,
                        perf_mode=mybir.MatmulPerfMode.DoubleRow,
                    )
                junk = dfpool.tile([128, 1024], BF16, tag="dfj")
                nc.scalar.activation(
                    junk[:],
                    dp[:],
                    mybir.ActivationFunctionType.Square,
                    accum_out=dstat[:, 4 * kk + q : 4 * kk + q + 1],
                )

            cones3 = cones[:].rearrange("p (two m) -> p two m", two=2)

            def emit_banded(s, u_tile, h2, last=False):
                nonlocal banded_emitted
                lo, hi = 64 - 2 * s, 192 - 2 * s
                first = banded_emitted == 0
                banded_emitted += 1
                # DoubleRow with W0 == W1: out[m, n] accumulates the column
                # pair u[p, n] + u[p, n+256]. The host supertile layout puts
                # column (s, t, j, i) = w-split pairs 256 apart, so the pair
                # is same-map: free index 512s + 256t + 32j + i, w = 32t + i.
                rhs = (
                    u_tile[:, 512 * h2 : 512 * (h2 + 1)]
                    .rearrange("p (two n) -> p two n", two=2)
                )
                nc.tensor.matmul(
                    sall[:],
                    cones3[:, :, lo:hi],
                    rhs,
                    start=first,
                    stop=last,
                    skip_group_check=True,
                    perf_mode=mybir.MatmulPerfMode.DoubleRow,
                )

            # DMA launches are spread across sync/gpsimd/scalar (parallel
            # descriptor generation — a single engine serializes launches at
            # ~600ns each) and prefetched 2 chunks ahead.
            ptiles, mtiles = {}, {}

            def launch(k):
                plane, c = k % 2, k // 2
                src = xh if plane == 0 else xt
                p_t = ipool.tile([128, CHUNK_F], FP8, tag="p")
                perm_t = ipool.tile([128, CHUNK_F], FP8, tag="perm")
                nc.sync.dma_start(p_t[:], src[c, 0])
                nc.sync.dma_start(perm_t[:], src[c, 1])
                mt = dpool.tile([128, 2 * CHUNK_F], FP8, tag="mt")
                nc.sync.dma_start(mt[:], md[k])
                ptiles[k] = (p_t, perm_t)
                mtiles[k] = mt

            # chunk 0's p/perm lead (they gate the first E-mm + stt), then the
            # tiny consts, then the rest — order matters: DMA queues are FIFO
            p0 = ipool.tile([128, CHUNK_F], FP8, tag="p")
            perm0 = ipool.tile([128, CHUNK_F], FP8, tag="perm")
            nc.sync.dma_start(p0[:], xh[0, 0])
            nc.sync.dma_start(perm0[:], xh[0, 1])
            nc.sync.dma_start(ce[:], c_e[:])
            if use_dr:
                nc.sync.dma_start(cw[:], c_w[:])
            nc.sync.dma_start(cones[:], c_ones[:])
            nc.sync.dma_start(ca[:], c_a[:])
            mt0 = dpool.tile([128, 2 * CHUNK_F], FP8, tag="mt")
            nc.sync.dma_start(mt0[:], md[0])
            ptiles[0] = (p0, perm0)
            mtiles[0] = mt0
            launch(1)

            for k in range(NCHUNK):
                plane = k % 2          # 0: h-major (xh), 1: w-major (xt)
                c = k // 2             # x0 chunk index
                if k + 2 < NCHUNK:
                    launch(k + 2)
                p_t, perm_t = ptiles.pop(k)
                mt = mtiles.pop(k)

                # data loss for this chunk
                sub = SUB_ENGINE[k]
                if sub == "dr":
                    # DoubleRow matmul subtract into PSUM (deferred — popped
                    # into the tensor stream during the next chunk's groups so
                    # the shared PSUM pool never gates fresh E-matmuls);
                    # scalar squares from PSUM per 1024 columns.
                    for q in range(4):
                        dr_q.append((mt, k, q))
                else:
                    # DVE runs the subtract as scalar_tensor_tensor (faster
                    # than tensor_tensor there); Pool can't run stt.
                    df = dfpool.tile([128, CHUNK_F], FP8, tag="df")
                    if sub == "dve":
                        nc.vector.scalar_tensor_tensor(
                            df[:],
                            mt[:, 0:CHUNK_F],
                            ca[:, 2:3],
                            mt[:, CHUNK_F : 2 * CHUNK_F],
                            op0=mybir.AluOpType.mult,
                            op1=mybir.AluOpType.subtract,
                        )
                    else:
                        nc.gpsimd.tensor_sub(
                            df[:], mt[:, 0:CHUNK_F], mt[:, CHUNK_F : 2 * CHUNK_F]
                        )
                    # split into halves so the scalar queue never blocks the
                    # shared PSUM pool for a full 4µs activation
                    for hh in range(2):
                        nc.scalar.activation(
                            df[:, 2048 * hh : 2048 * (hh + 1)],
                            df[:, 2048 * hh : 2048 * (hh + 1)],
                            mybir.ActivationFunctionType.Square,
                            accum_out=dstat[:, 4 * k + hh : 4 * k + hh + 1],
                        )

                # residual: 8 supertiles per chunk in pairs; both planes
                # accumulate into the same PSUM rows (identical batch mapping).
                for g in range(4):
                    ep = pwpool.tile([128, 1024], F32, tag="ep")
                    for h2 in range(2):
                        t = 2 * g + h2
                        nc.tensor.matmul(
                            ep[:, 512 * h2 : 512 * (h2 + 1)],
                            ce[:],
                            p_t[:, 512 * t : 512 * (t + 1)],
                            start=True,
                            stop=True,
                        )
                    u = wpool.tile([128, 1024], FP8, tag="u")
                    nc.vector.scalar_tensor_tensor(
                        u[:],
                        perm_t[:, 1024 * g : 1024 * (g + 1)],
                        ca[:, plane : plane + 1],
                        ep[:],
                        op0=mybir.AluOpType.add,
                        op1=mybir.AluOpType.mult,
                    )
                    for h2 in range(2):
                        banded_q.append((8 * c + 2 * g + h2, u, h2))
                    while len(banded_q) > BANDED_LAG:
                        emit_banded(*banded_q.pop(0))
                    if dr_q and dr_q[0][1] < k:
                        emit_dr(*dr_q.pop(0))

            while dr_q:
                emit_dr(*dr_q.pop(0))
            while banded_q:
                ent = banded_q.pop(0)
                emit_banded(*ent, last=not banded_q)

            s_t = stpool.tile([64, 8], F32, tag="st")
            nc.vector.reduce_sum(
                s_t[:],
                sall[0:64, :].rearrange("p (j w) -> p j w", j=8),
                axis=mybir.AxisListType.X,
            )  # 32 pair-columns per map
            nc.sync.dma_start(s_out[:], s_t[:])
            nc.sync.dma_start(dstat_out[:], dstat[:])

    nc.compile()
    return nc


_NC = None
_CONSTS = None
LAST_RESULTS = None


def kernel(model_out, target, x0_hat, var, _trace=False, _trace_kwargs=None):
    global _NC, _CONSTS, LAST_RESULTS
    if _NC is None:
        _CONSTS = _build_consts()
        _NC = _build_nc()

    import ml_dtypes

    f8 = ml_dtypes.float8_e4m3
    model_out = np.asarray(model_out).astype(f8)
    target = np.asarray(target).astype(f8)
    x0f = np.asarray(x0_hat, dtype=np.float32).copy()
    x0f[:, 1] *= 1.0 / PSCALE  # keep fp8 u in range
    x0_hat = x0f.astype(f8)
    var = np.asarray(var, dtype=np.float32)

    in_maps = []
    for cid in range(N_CORES):
        lo, hi = cid * BPC, (cid + 1) * BPC
        # (s, r, j, ch, h, t, i) with batch = 16s + 8r + j and w = 32t + i;
        # free index is 512s + 256t + 32j + i so same-map column pairs sit
        # 256 apart (DoubleRow banded pair-sum).
        x0c = x0_hat[lo:hi].reshape(NS, 2, 8, 2, H, 2, 32)
        # h-major: [ch, 64r+h, 512s+256t+32j+i]
        xh_arr = (
            x0c.transpose(3, 1, 4, 0, 5, 2, 6)
            .reshape(2, 128, NXCH, CHUNK_F)
            .transpose(2, 0, 1, 3)
            .copy()
        )
        # w-major (per-map transpose): [ch, 64r+w, 512s+256th+32j+ih],
        # h = 32*th + ih
        x0t = x0_hat[lo:hi].reshape(NS, 2, 8, 2, 2, 32, W)
        xt_arr = (
            x0t.transpose(3, 1, 6, 0, 4, 2, 5)
            .reshape(2, 128, NXCH, CHUNK_F)
            .transpose(2, 0, 1, 3)
            .copy()
        )
        md_arr = np.concatenate(
            [
                model_out[lo:hi].reshape(NCHUNK, 128, CHUNK_F),
                target[lo:hi].reshape(NCHUNK, 128, CHUNK_F),
            ],
            axis=-1,
        )
        consts = dict(_CONSTS)
        if "dr" not in SUB_ENGINE:
            consts.pop("cW", None)
        in_maps.append({"xh": xh_arr, "xt": xt_arr, "md": md_arr, **consts})

    kwargs = {}
    if _trace:
        kwargs["trace"] = True
        if _trace_kwargs:
            kwargs.update(_trace_kwargs)
    res = run_bass_kernel_spmd(_NC, in_maps, list(range(N_CORES)), **kwargs)
    LAST_RESULTS = res

    data_sum = 0.0
    nll_sum = 0.0
    for cid in range(N_CORES):
        out = res.results[cid]
        s1 = out["s"].astype(np.float64)         # [64, 8]
        dstat = out["dstat"][:, : 4 * NCHUNK].astype(np.float64)

        # s1[2s+r, j] -> batch 16s + 8r + j
        r = s1.reshape(NS, 2, 8).reshape(BPC) * (ESCALE * PSCALE / (H * W * 3.0))

        v = var[cid * BPC : (cid + 1) * BPC].astype(np.float64)
        nll = np.minimum(0.5 * r * r / v, CLAMP_NEG_MIN)
        nll_sum += nll.sum()
        data_sum += dstat.sum()

    loss = data_sum / (B * C * H * W) + nll_sum / B
    return np.float32(loss)
